# revision 58
# baseline (speedup 1.0000x reference)
"""Trainium2 Bass kernel for nn_DecoderBlock (B=4, S=1024, H=1024, 16 heads).

Sharding (8 cores, zero cross-core communication):
  core c -> batch b = c//2, half = c%2; own query rows are the interleaved
  256-row blocks {B0,B3} (half 0) / {B1,B2} (half 1) -- see own_rows().
  Row-parallel over the sequence for LN / projections / FFN; each core of a
  batch pair duplicates the full K/V projections (they depend only on the
  INPUTS x / key_and_value, never on the other core's partial results).

Device layout strategy:
  - CROSS-ATTENTION projections (q2/k2/v2/wo2) run in fp8e4m3 DoubleRow
    perf mode: weights host-quantized at 32x, both matmul operands fp8,
    two k-tiles contracted per instruction -> 2x+ PE throughput.  The
    fp8 inputs are ln2kvT / lnz8 / ctxT2, quantized for free at their
    PSUM evictions.  Numerics validated host-side: CA-only fp8 adds
    ~5e-3 rel err (gate 2e-2); SA / FFN stay fp16 (their fp8 error is
    10x larger: causal short rows / 2-layer amplification).
  - Scale flow: residual stream carries 1024x (x_own*1024, sa_wo*1024,
    ca 32*32, fc_w2*1024 -- all folded host-side; CA exp scale 2^-13;
    final output divided by 1024 on the host).  No extra device ops.
  - Attention in transposed-score space: scoresT[k, q] = kT.T @ qT per head,
    exp on ACT, causal/padding mask as a 0/1 fp16 MULTIPLY on the exp output,
    softmax denominator via a ones-column appended to V, normalized with
    reciprocal + gpsimd partition_broadcast.  ctxT feeds the output
    projection directly as lhsT.
  - Rows host-permuted own-first; interleaved block sharding makes the
    causally-clipped SA_SCHED (75% of the rectangle, the optimal union
    across the core pair) valid for BOTH cores with one uniform program.

Engine balance:
  - LN stats are BATCHED per layernorm_T half (4 chunks): one Ln + one Exp
    per batch instead of per chunk kills the ACT table-set ping-pong
    between natural_log (set 5) and exp_and_others (set 0): 38 ->
    ~12 LoadActFuncSet (1.28us each, and each stalls the eviction copies
    queued behind it on ACT).
  - rstd = exp(-0.5*ln(var+eps)) on ACT; LN transpose evictions on ACT;
    projection evictions on DVE.
  - qTp zero padding written once at build time, on GPSIMD (Pool) so the
    DVE is free for the first LN stats during the input DMA window.
  - v_aug is [v(64)|ones] (65-wide lhsT slice).
  - kv LN emitted inside the SA projection window.
"""

import sys

sys.path.insert(0, "/opt/trn_rl_repo")

import time
from contextlib import ExitStack

import numpy as np
import ml_dtypes

import concourse.bass as bass
import concourse.mybir as mybir
import concourse.tile as tile
from concourse import bacc
from concourse.masks import make_identity

F32 = mybir.dt.float32
F16 = mybir.dt.float16
F8 = mybir.dt.float8e4
AF = mybir.ActivationFunctionType
OP = mybir.AluOpType
DR = mybir.MatmulPerfMode.DoubleRow

B, S, H, NH, DK, FF = 4, 1024, 1024, 16, 64, 4096
P = 128
HT = H // P  # 8 feature tiles of the model dim
QN = 512  # own query rows per core
QC = QN // P  # 4 query chunks
RC = S // P  # 8 key/row chunks
FT = FF // P  # 32 ffn tiles
NEG = -50000.0
VW = 72  # per-head width of v_aug: [v(64) | ones | 7 pad cols never touched]
EPS = 1e-5
OUT_SCALE = 1024.0  # residual stream scale; host divides the output
WQ8 = 32.0  # fp8 weight quantization scale (CA projections)
E4NP = ml_dtypes.float8_e4m3

_CACHE: dict = {}
LAST_RUN_NS: int | None = None


def _ilv(k):
    """Row shuffle so that SBUF [P, k, n] loaded with "(p k) n -> p k n"
    (contiguous k*rowbytes per partition) holds orig row 128*c+p at
    (partition p, chunk c):  shuf[k*p + c] = orig[128*c + p]."""
    return (np.arange(k)[None, :] * 128 + np.arange(128)[:, None]).reshape(-1)


IDX8 = _ilv(8)
IDX4 = _ilv(4)
IDX32 = _ilv(32)


def _bcast_row_ap(dram_ap, parts=P):
    """DRAM [1, N] -> partition-broadcast AP [parts, N] (step-0 partition dim)."""
    return bass.AP(
        tensor=dram_ap.tensor, offset=dram_ap.offset, ap=[[0, parts], dram_ap.ap[1]]
    )


def _build(flags: frozenset, repeat: int = 1, ablate: frozenset = frozenset()):
    """Build + compile the single SPMD program. `flags` toggles optional ops."""
    use_ca_mask = "ca_mask" in flags
    use_sa_full = "sa_full" in flags
    ln_g = {i: f"ln{i}_g" in flags for i in (1, 2, 3)}
    ln_b = {i: f"ln{i}_b" in flags for i in (1, 2, 3)}
    use_b2 = "b2" in flags

    nc = bacc.Bacc("TRN2", target_bir_lowering=False, debug=False, num_devices=8)

    D = {}

    def din(name, shape, dt):
        D[name] = nc.dram_tensor(name, shape, dt, kind="ExternalInput").ap()

    din("x_own", [QN, H], F16)  # 1024x scaled
    din("x_rm", [S, H], F16)  # permuted rows (own first), unscaled
    din("kv_rm", [S, H], F16)
    din("sa_cb", [P, RC, QN], F16)  # 0/1 mask, permuted key order
    if use_ca_mask:
        din("ca_cb", [P, RC, QN], F16)
    for w in ("sa_wv", "sa_wo"):
        din(w, [H, H], F16)  # sa_wo carries 1024x
    for w in ("sa_wqhl", "sa_wkhl"):
        din(w, [2 * H, H], F8)  # 32x quantized [hi | lo] halves, compensated
    for w in ("ca_wq", "ca_wk", "ca_wv", "ca_wo"):
        din(w, [H, H], F8)  # 32x quantized
    din("fc_w1", [H, FF], F16)
    din("fc_w2", [FF, H], F16)  # 1024x
    din("fc_b1f", [P, FT], F32)  # feature-major b1
    if use_b2:
        din("fc_b2", [1, H], F32)  # 1024x
    for i in (1, 2, 3):
        if ln_g[i]:
            din(f"ln{i}_g", [1, H], F32)
        if ln_b[i]:
            din(f"ln{i}_b", [1, H], F32)
    out_d = nc.dram_tensor("out", [QN, H], F16, kind="ExternalOutput").ap()

    with tile.TileContext(nc) as tc, ExitStack() as top:
        sing = top.enter_context(tc.tile_pool(name="sing", bufs=1))
        ident = sing.tile([P, P], F16)
        make_identity(nc, ident)
        eps_t = sing.tile([P, 1], F32)
        nc.vector.memset(eps_t, EPS)
        ones_c = sing.tile([P, 1], F32)
        nc.vector.memset(ones_c, 1.0)

        # broadcast [1,H] params across partitions via step-0 DMA
        gb_tiles = {}
        for i in (1, 2, 3):
            for kind, on in (("g", ln_g[i]), ("b", ln_b[i])):
                if on:
                    t = sing.tile([P, H], F32)
                    nc.sync.dma_start(out=t, in_=_bcast_row_ap(D[f"ln{i}_{kind}"]))
                    gb_tiles[(i, kind)] = t
        b1f = sing.tile([P, FT], F32)
        nc.sync.dma_start(out=b1f, in_=D["fc_b1f"])
        b2_t = None
        if use_b2:
            b2_t = sing.tile([P, H], F32)
            nc.sync.dma_start(out=b2_t, in_=_bcast_row_ap(D["fc_b2"]))

        small = top.enter_context(tc.tile_pool(name="small", bufs=4))
        norms = top.enter_context(tc.tile_pool(name="norms", bufs=2))
        scratch = top.enter_context(tc.tile_pool(name="scratch", bufs=2))
        lnzp = top.enter_context(tc.tile_pool(name="lnzp", bufs=1))

        # persistent q tile: zero halves written once (GPSIMD: DVE is needed
        # for the first LN stats during the initial DMA window).
        qTp = sing.tile([P, NH, QN], F16, tag="qTp")
        nc.gpsimd.memset(qTp, 0.0)

        def emit_once():
         # x/kv DMAs are emitted FIRST (inside the acts block below): LN1 is
         # the program's critical path.  x_own (SA residual, needed ~100us
         # in) and the masks (needed at SA attention) queue after them.
         xown16 = sing.tile([P, QC, H], F16, tag="xown")
         sacb = sing.tile([P, RC, QN], F16, tag="sacb")
         cacb = None
         if use_ca_mask:
             cacb = sing.tile([P, RC, QN], F16, tag="cacb")
         z = sing.tile([P, QC, H], F16, tag="z")
         z2 = sing.tile([P, QC, H], F16, tag="z2")

         # ps_mm is reopened per projection phase (closed across the
         # attention blocks, whose score/AV pools need 6 PSUM banks):
         # bufs=4 gives the projection chains twice the pipeline depth a
         # program-wide pool could afford.
         mm_stack = ExitStack()
         ps_mm = mm_stack.enter_context(
             tc.tile_pool(name="ps_mm1", bufs=4, space="PSUM")
         )

         def layernorm_T(row_ap_fn, n_rc, lnT, ln_idx, ps_t, tag, out_dt=F16,
                         lnT8=None, batches=None):
            """Row-major LN stats+apply, then PE-transpose into feature-major lnT.

            Stats are batched in halves (<=4 chunks) so ACT runs ONE Ln and
            ONE Exp per batch -- avoids the per-chunk table-set ping-pong.
            Transposes always run fp16 (walrus rejects fp8 transpose with a
            unit-stride output); the ACT eviction converts to out_dt.
            """
            idt = ident
            if batches is None:
                batches = [min(4, n_rc - r) for r in range(0, n_rc, 4)]
            rc0 = 0
            for nb in batches:
                mvall = small.tile([P, nb, 2], F32, tag="mva")
                for j in range(nb):
                    row = row_ap_fn(rc0 + j)
                    st = small.tile([P, 2, 6], F32, tag="st")
                    nc.vector.bn_stats(out=st[:, 0], in_=row[:, 0:512])
                    nc.vector.bn_stats(out=st[:, 1], in_=row[:, 512:H])
                    nc.vector.bn_aggr(out=mvall[:, j], in_=st)
                # rstd = exp(-0.5*ln(var+eps)): keeps ACT on the table sets
                # that also serve Exp/Copy/Relu, one load per batch.
                lnv = small.tile([P, nb], F32, tag="lnv")
                nc.scalar.activation(
                    out=lnv, in_=mvall[:, :, 1:2], func=AF.Ln, bias=eps_t, scale=1.0
                )
                rstd = small.tile([P, nb], F32, tag="rstd")
                nc.scalar.activation(out=rstd, in_=lnv, func=AF.Exp, scale=-0.5)
                for j in range(nb):
                    rc = rc0 + j
                    row = row_ap_fn(rc)
                    lnr = scratch.tile([P, H], F16, tag="lnr")
                    nc.vector.tensor_scalar(
                        out=lnr,
                        in0=row,
                        scalar1=mvall[:, j, 0:1],
                        scalar2=rstd[:, j : j + 1],
                        op0=OP.subtract,
                        op1=OP.mult,
                    )
                    if (ln_idx, "g") in gb_tiles:
                        nc.vector.tensor_mul(
                            out=lnr, in0=lnr, in1=gb_tiles[(ln_idx, "g")]
                        )
                    if (ln_idx, "b") in gb_tiles:
                        nc.vector.tensor_add(
                            out=lnr, in0=lnr, in1=gb_tiles[(ln_idx, "b")]
                        )
                    for f in range(HT):
                        pt = ps_t.tile([P, P], F16, tag="pt")
                        nc.tensor.transpose(pt, lnr[:, f * P : (f + 1) * P], idt)
                        # evict on ACT: DVE is busy with LN stats/apply here
                        nc.scalar.copy(
                            out=lnT[:, f, rc * P : (rc + 1) * P], in_=pt
                        )
                        if lnT8 is not None:
                            # fp8 shadow for DoubleRow consumers (DVE)
                            nc.vector.tensor_copy(
                                out=lnT8[:, f, rc * P : (rc + 1) * P], in_=pt
                            )
                rc0 += nb

         _w8_cache = {}

         def load_w8(dram, pool, dt=F16, kt=HT):
            if "nowdma" in ablate:
                if "w8" not in _w8_cache:
                    t = pool.tile([P, HT, H], F16, tag="w8")
                    nc.sync.dma_start(
                        out=t, in_=dram.rearrange("(p k) n -> p k n", p=P)
                    )
                    _w8_cache["w8"] = t
                return _w8_cache["w8"]
            t = pool.tile([P, kt, H], dt, tag="w8")
            nc.sync.dma_start(out=t, in_=dram.rearrange("(p k) n -> p k n", p=P))
            return t

         FULL_SCHED = ((512, 0),) * RC
         # uniform causally-clipped SA schedule (see _prep_core block layout):
         # kc pairs alternate full-width and upper-half-only (q cols 256..511)
         SA_SCHED = (
             (512, 0), (512, 0), (256, 256), (256, 256),
             (512, 0), (512, 0), (256, 256), (256, 256),
         )

         def attention(qT, kT, v_aug, ctxT, cb, ps_s, ps_av, exp_pool, sched,
                       exp_scale=0.125):
            for h in range(NH):
                f, r0 = h // 2, (h % 2) * 64
                pm_av = (
                    None
                    if "noav" in ablate
                    else ps_av.tile([P, QN], F32, tag="av")
                )
                av_out = None if pm_av is None else pm_av[0:65]
                for g in range(4):
                    n, off = sched[2 * g]  # both kc of a pair share (n, off)
                    et = exp_pool.tile([P, 2, QN], F16, tag="et")
                    if "noscores" in ablate:
                        nc.vector.memset(et, 0.01)
                    else:
                        pm_s = ps_s.tile([P, 2, QN], F32, tag="s")
                        for j in range(2):
                            kc = g * 2 + j
                            # K=128 contraction: other head's rows in qT are 0
                            nc.tensor.matmul(
                                pm_s[:, j, 0:n],
                                lhsT=kT[:, f, kc * P : (kc + 1) * P],
                                rhs=qT[:, h, off : off + n],
                                start=True,
                                stop=True,
                            )
                        if "noexp" in ablate:
                            nc.vector.tensor_copy(
                                out=et[:, :, 0:n], in_=pm_s[:, :, 0:n]
                            )
                        else:
                            nc.scalar.activation(
                                out=et[:, :, 0:n],
                                in_=pm_s[:, :, 0:n],
                                func=AF.Exp,
                                scale=exp_scale,
                            )
                        if cb is not None and "nobias" not in ablate:
                            # exp(s)*m == masked softmax numerator (m in {0,1})
                            nc.vector.tensor_mul(
                                out=et[:, :, 0:n],
                                in0=et[:, :, 0:n],
                                in1=cb[:, 2 * g : 2 * g + 2, off : off + n],
                            )
                    if "noav" not in ablate:
                        for j in range(2):
                            kc = g * 2 + j
                            # lhsT = [v(64) | ones]: psum row 64 accumulates
                            # sumexp; only psum rows 0:65 are written/read.
                            nc.tensor.matmul(
                                av_out[:, off : off + n],
                                lhsT=v_aug[:, kc, h, 0:65],
                                rhs=et[:, j, 0:n],
                                start=(kc == 0),
                                stop=(kc == RC - 1),
                            )
                if "noav" in ablate:
                    nc.vector.memset(ctxT[r0 : r0 + 64, f, :], 0.25)
                elif "nonorm" in ablate:
                    nc.vector.tensor_copy(
                        out=ctxT[r0 : r0 + 64, f, :], in_=pm_av[0:64, :]
                    )
                else:
                    recip = norms.tile([1, QN], F16, tag="rec")
                    with nc.allow_low_precision(
                        reason="1/sumexp in fp16 is within output tolerance"
                    ):
                        nc.vector.reciprocal(out=recip, in_=pm_av[64:65, :])
                    rb = norms.tile([64, QN], F16, tag="rb")
                    nc.gpsimd.partition_broadcast(out_ap=rb, in_ap=recip)
                    nc.vector.tensor_mul(
                        out=ctxT[r0 : r0 + 64, f, :], in0=pm_av[0:64, :], in1=rb
                    )

         def mm_acc(pm, w_sb, rhs_fn, dr, w_lo=None):
            """PSUM-accumulated contraction over HT k-tiles; DoubleRow pairs
            adjacent k-tiles when dr (operands must be fp8).  With w_lo, a
            second DR chain accumulates the quantization residual of the
            weights (2x instead of 4x, but weight error ~0.1% not 3%)."""
            if dr:
                for kp in range(HT // 2):
                    nc.tensor.matmul(
                        pm,
                        lhsT=w_sb(2 * kp, 2),
                        rhs=rhs_fn(2 * kp, 2),
                        start=(kp == 0),
                        stop=(kp == HT // 2 - 1 and w_lo is None),
                        perf_mode=DR,
                    )
                    if w_lo is not None:
                        nc.tensor.matmul(
                            pm,
                            lhsT=w_lo(2 * kp, 2),
                            rhs=rhs_fn(2 * kp, 2),
                            start=False,
                            stop=(kp == HT // 2 - 1),
                            perf_mode=DR,
                        )
            else:
                for kc in range(HT):
                    nc.tensor.matmul(
                        pm,
                        lhsT=w_sb(kc, 1),
                        rhs=rhs_fn(kc, 1),
                        start=(kc == 0),
                        stop=(kc == HT - 1),
                    )

         def proj_heads_qpad(qT_pad, w_sb, lnT, dr=False, w_lo=None):
            # qT_pad[:, h, :]: head h q-dims at rows (h%2)*64..+64, other 64
            # rows zero (zeroed once at build time -- qT_pad is qTp).
            for f in range(HT):
                pm = ps_mm.tile([P, 512], F32, tag="proj")
                mm_acc(
                    pm,
                    lambda k, n: w_sb[:, k : k + n, f * P : (f + 1) * P],
                    lambda k, n: lnT[:, k : k + n, 0:QN],
                    dr,
                    w_lo=None
                    if w_lo is None
                    else (lambda k, n: w_lo[:, k : k + n, f * P : (f + 1) * P]),
                )
                nc.vector.tensor_copy(out=qT_pad[0:64, 2 * f, :], in_=pm[0:64, :])
                nc.vector.tensor_copy(
                    out=qT_pad[64:128, 2 * f + 1, :], in_=pm[64:128, :]
                )

         def proj_to_featmajor(outT, w_sb, lnT, n_cols, dr=False, w_lo=None):
            # outT[:, f, c*512:+512] = sum_kc w[kc,f]^T @ lnT[kc, cols]
            for f in range(HT):
                for c in range(n_cols // 512):
                    pm = ps_mm.tile([P, 512], F32, tag="proj")
                    mm_acc(
                        pm,
                        lambda k, n: w_sb[:, k : k + n, f * P : (f + 1) * P],
                        lambda k, n: lnT[:, k : k + n, c * 512 : (c + 1) * 512],
                        dr,
                        w_lo=None
                        if w_lo is None
                        else (lambda k, n: w_lo[:, k : k + n, f * P : (f + 1) * P]),
                    )
                    nc.vector.tensor_copy(
                        out=outT[:, f, c * 512 : (c + 1) * 512], in_=pm
                    )

         def make_v_aug(v_aug, w_sb, lnT, dr=False):
            # cols 65:VW are never written nor read (AV lhsT slices 0:65)
            nc.vector.tensor_copy(
                out=v_aug[:, :, :, 64:65], in_=ones_c.to_broadcast([P, RC, NH, 1])
            )
            for kc in range(RC):
                for vc in range(2):
                    pm = ps_mm.tile([P, 512], F32, tag="proj")
                    mm_acc(
                        pm,
                        lambda k, n: lnT[:, k : k + n, kc * P : (kc + 1) * P],
                        lambda k, n: w_sb[:, k : k + n, vc * 512 : (vc + 1) * 512],
                        dr,
                    )
                    nc.vector.tensor_copy(
                        out=v_aug[:, kc, vc * 8 : (vc + 1) * 8, 0:64],
                        in_=pm.rearrange("p (h d) -> p h d", h=8),
                    )

         def wo_residual(ctxT, w_sb, base, out_rows, dr=False):
            # out_rows[:, qc, :] = base[:, qc, :] + ctx @ wo
            for qc in range(QC):
                for ncol in range(2):
                    pm = ps_mm.tile([P, 512], F32, tag="proj")
                    mm_acc(
                        pm,
                        lambda k, n: ctxT[:, k : k + n, qc * P : (qc + 1) * P],
                        lambda k, n: w_sb[:, k : k + n, ncol * 512 : (ncol + 1) * 512],
                        dr,
                    )
                    sl = slice(ncol * 512, (ncol + 1) * 512)
                    nc.vector.tensor_tensor(
                        out=out_rows[:, qc, sl], in0=pm, in1=base[:, qc, sl], op=OP.add
                    )

         lnz = lnzp.tile([P, HT, QN], F16, tag="lnzT")
         lnz8 = lnzp.tile([P, HT, QN], F8, tag="lnz8T")

         with tc.tile_pool(name="attn_acts", bufs=1) as acts, tc.tile_pool(
            name="wpool", bufs=2
         ) as wpool:
            # ---------------- P0: LN1(x) -> ln1T ----------------
            # quarter the DMA so LN of the first chunks starts early
            x_sb = acts.tile([P, RC, H], F16, tag="kT")
            x_ap = D["x_rm"].rearrange("(p k) n -> p k n", p=P)
            for i in range(4):
                nc.sync.dma_start(
                    out=x_sb[:, 2 * i : 2 * i + 2, :], in_=x_ap[:, 2 * i : 2 * i + 2, :]
                )
            ln1_stack = ExitStack()
            ln1p = ln1_stack.enter_context(tc.tile_pool(name="ln1p", bufs=1))
            ln1T = ln1p.tile([P, HT, S], F16, tag="lnT")
            ln1T8 = ln1p.tile([P, HT, S], F8, tag="lnT8")
            ln2kvT = acts.tile([P, HT, S], F8, tag="lnT2")
            kv_stack = ExitStack()
            kvp = kv_stack.enter_context(tc.tile_pool(name="kvp", bufs=1))
            kv_sb = kvp.tile([P, RC, H], F16, tag="kvrows")
            kv_ap = D["kv_rm"].rearrange("(p k) n -> p k n", p=P)
            nc.sync.dma_start(out=kv_sb[:, 0:4, :], in_=kv_ap[:, 0:4, :])
            nc.sync.dma_start(out=kv_sb[:, 4:8, :], in_=kv_ap[:, 4:8, :])
            # non-critical inputs queue after x/kv
            nc.sync.dma_start(
                out=xown16, in_=D["x_own"].rearrange("(p q) n -> p q n", p=P)
            )
            nc.sync.dma_start(out=sacb, in_=D["sa_cb"])
            if use_ca_mask:
                nc.sync.dma_start(out=cacb, in_=D["ca_cb"])
            with tc.tile_pool(name="ps_t1", bufs=2, space="PSUM") as ps_t:
                # first batch of 2 so the first transposes (the program's
                # first PE work) start as soon as 2 x-chunks have landed
                layernorm_T(
                    lambda rc: x_sb[:, rc, :], RC, ln1T, 1, ps_t, "l1",
                    lnT8=ln1T8, batches=(2, 3, 3),
                )

                # ------------- P1: SA projections -------------
                # kv LN is emitted between the projections: its DVE/ACT work
                # fills the PE-bound projection window, keeping it off the
                # critical path well before CA needs it.  Its fp8 output is
                # the CA K/V DoubleRow operand.
                # SA q/k run compensated DoubleRow (hi+lo fp8 weights, fp8
                # ln1T shadow): 2x PE, ~fp16 weight precision.
                qT = qTp
                kT = acts.tile([P, HT, S], F16, tag="kT")
                v_aug = acts.tile([P, RC, NH, VW], F16, tag="vaug")
                wqhl = load_w8(D["sa_wqhl"], wpool, dt=F8, kt=2 * HT)
                proj_heads_qpad(
                    qT, wqhl[:, 0:HT], ln1T8, dr=True, w_lo=wqhl[:, HT : 2 * HT]
                )
                wkhl = load_w8(D["sa_wkhl"], wpool, dt=F8, kt=2 * HT)
                proj_to_featmajor(
                    kT, wkhl[:, 0:HT], ln1T8, S, dr=True, w_lo=wkhl[:, HT : 2 * HT]
                )
                layernorm_T(
                    lambda rc: kv_sb[:, rc, :], RC, ln2kvT, 2, ps_t, "l2kv", out_dt=F8
                )
                wv = load_w8(D["sa_wv"], wpool)
                make_v_aug(v_aug, wv, ln1T)
            kv_stack.close()  # kv rows dead; frees 16KB/p for the exp pools
            ln1_stack.close()  # ln1 outputs dead; frees 24KB/p
            mm_stack.close()  # free proj PSUM banks for the attention pools

            # ---------------- SA attention ----------------
            ctxT = acts.tile([P, HT, QN], F16, tag="ctxT")
            if "noattn" in ablate:
                nc.vector.memset(ctxT, 0.25)
            else:
                with (
                    tc.tile_pool(name="ps_s1", bufs=3, space="PSUM") as ps_s,
                    tc.tile_pool(name="ps_av1", bufs=2, space="PSUM") as ps_av,
                    tc.tile_pool(name="exp1", bufs=4) as exp_pool,
                ):
                    attention(
                     qT, kT, v_aug, ctxT, sacb, ps_s, ps_av, exp_pool,
                     FULL_SCHED if use_sa_full else SA_SCHED,
                     exp_scale=0.125 / 1024.0,  # q,k carry 32x each
                 )

            # ---------------- SA wo + residual -> z ----------------
            ps_mm = mm_stack.enter_context(
                tc.tile_pool(name="ps_mm2", bufs=4, space="PSUM")
            )
            wo = load_w8(D["sa_wo"], wpool)
            wo_residual(ctxT, wo, xown16, z)

            # ---------------- P2: cross attention (fp8 DoubleRow) --------
            # CA K/V projections depend only on ln2kvT (ready since P0) and
            # fill PE while SA attention is ACT(exp)-bound.
            cwk = load_w8(D["ca_wk"], wpool, dt=F8)
            kT2 = acts.tile([P, HT, S], F16, tag="kT")
            proj_to_featmajor(kT2, cwk, ln2kvT, S, dr=True)
            cwv = load_w8(D["ca_wv"], wpool, dt=F8)
            v_aug2 = acts.tile([P, RC, NH, VW], F16, tag="vaug")
            make_v_aug(v_aug2, cwv, ln2kvT, dr=True)

            with tc.tile_pool(name="ps_t2", bufs=2, space="PSUM") as ps_t:
                layernorm_T(
                    lambda rc: z[:, rc, :], QC, lnz8, 2, ps_t, "l2z", out_dt=F8
                )
            qT2 = qTp
            cwq = load_w8(D["ca_wq"], wpool, dt=F8)
            proj_heads_qpad(qT2, cwq, lnz8, dr=True)
            mm_stack.close()  # free proj PSUM banks for the CA attention pools

            ctxT2 = acts.tile([P, HT, QN], F8, tag="ctxT")  # reuses SA ctx slot
            if "noattn" in ablate:
                nc.vector.memset(ctxT2, 0.25)
            else:
                with (
                    tc.tile_pool(name="ps_s2", bufs=3, space="PSUM") as ps_s,
                    tc.tile_pool(name="ps_av2", bufs=2, space="PSUM") as ps_av,
                    tc.tile_pool(name="exp2", bufs=4) as exp_pool,
                ):
                    attention(
                        qT2, kT2, v_aug2, ctxT2, cacb, ps_s, ps_av, exp_pool,
                        FULL_SCHED, exp_scale=0.125 / 1024.0,
                    )

            ps_mm = mm_stack.enter_context(
                tc.tile_pool(name="ps_mm3", bufs=4, space="PSUM")
            )
            cwo = load_w8(D["ca_wo"], wpool, dt=F8)
            wo_residual(ctxT2, cwo, z, z2, dr=True)

         # ---------------- P3: FFN ----------------
         with tc.tile_pool(name="ps_t3", bufs=2, space="PSUM") as ps_t:
            layernorm_T(lambda rc: z2[:, rc, :], QC, lnz, 3, ps_t, "l3")

         with (
             tc.tile_pool(name="hTpool", bufs=1) as hTpool,
             tc.tile_pool(name="w2pool", bufs=3) as w2pool,
         ):
            hT = hTpool.tile([P, FT, QN], F16, tag="hT")
            w2_ap = D["fc_w2"].rearrange("(p k) n -> p k n", p=P)
            mm_stack.close()  # all 8 PSUM banks for the staged w1 chains
            with tc.tile_pool(name="w1pool", bufs=1) as w1pool:
                # fc_w1 staged as 4 x 2MB so the first chains start after
                # 2MB instead of stalling mid-chain on the full 8MB; the
                # stage-major loop keeps 8 PSUM chains open (one per ft of
                # the group) so compute streams behind the DMA.
                w1_ap = D["fc_w1"].rearrange("(p k) n -> p k n", p=P)
                w1c = []
                for s in range(4):
                    t = w1pool.tile([P, 2, FF], F16, tag=f"w1c{s}")
                    nc.sync.dma_start(out=t, in_=w1_ap[:, 2 * s : 2 * s + 2, :])
                    w1c.append(t)
                # hoist the first w2 tile load so the w2 phase starts hot
                w2t0 = w2pool.tile([P, 4, H], F16, tag="w2s")
                nc.sync.dma_start(out=w2t0, in_=w2_ap[:, 0:4, :])
                with tc.tile_pool(name="ps_w1", bufs=8, space="PSUM") as ps_w1:
                    for g in range(4):
                        pms = [
                            ps_w1.tile([P, 512], F32, tag="w1ps", name=f"w1ps_{g}_{i}")
                            for i in range(8)
                        ]
                        for s in range(4):
                            for fti in range(8):
                                ft = g * 8 + fti
                                for j in range(2):
                                    kc = 2 * s + j
                                    nc.tensor.matmul(
                                        pms[fti],
                                        lhsT=w1c[s][:, j, ft * P : (ft + 1) * P],
                                        rhs=lnz[:, kc, :],
                                        start=(kc == 0),
                                        stop=(kc == HT - 1),
                                    )
                        for fti in range(8):
                            nc.scalar.activation(
                                out=hT[:, g * 8 + fti, :],
                                in_=pms[fti],
                                func=AF.Relu,
                                bias=b1f[:, g * 8 + fti : g * 8 + fti + 1],
                            )
            with tc.tile_pool(name="ps_big", bufs=1, space="PSUM") as ps_big:
                pm8 = ps_big.tile([P, 8, 512], F32)
                w2t = None
                for kc in range(FT):
                    if kc % 4 == 0:
                        if kc == 0:
                            w2t = w2t0  # prefetched during the hT phase
                        else:
                            w2t = w2pool.tile([P, 4, H], F16, tag="w2s")
                            nc.sync.dma_start(
                                out=w2t, in_=w2_ap[:, kc : kc + 4, :]
                            )
                    for qc in range(QC):
                        for ncol in range(2):
                            nc.tensor.matmul(
                                pm8[:, qc * 2 + ncol, :],
                                lhsT=hT[:, kc, qc * P : (qc + 1) * P],
                                rhs=w2t[:, kc % 4, ncol * 512 : (ncol + 1) * 512],
                                start=(kc == 0),
                                stop=False,
                            )
                out_ap = out_d.rearrange("(q p) n -> p q n", p=P)
                out_rows = sing.tile([P, QC, H], F16, tag="z")  # reuses z slot
                for qc in range(QC):
                    for ncol in range(2):
                        sl = slice(ncol * 512, (ncol + 1) * 512)
                        # residual add ON PE (psum += I.T @ z2): the tail
                        # eviction becomes a pure copy, split ACT/DVE so the
                        # two halves drain in parallel.
                        nc.tensor.matmul(
                            pm8[:, qc * 2 + ncol, :],
                            lhsT=ident,
                            rhs=z2[:, qc, sl],
                            start=False,
                            stop=True,
                        )
                        if b2_t is not None:
                            nc.vector.tensor_tensor(
                                out=out_rows[:, qc, sl],
                                in0=pm8[:, qc * 2 + ncol, :],
                                in1=b2_t[:, sl],
                                op=OP.add,
                            )
                        elif ncol == 0:
                            nc.scalar.copy(
                                out=out_rows[:, qc, sl], in_=pm8[:, qc * 2, :]
                            )
                        else:
                            nc.vector.tensor_copy(
                                out=out_rows[:, qc, sl],
                                in_=pm8[:, qc * 2 + 1, :],
                            )
                    # stream each query chunk out as soon as it is complete
                    nc.sync.dma_start(
                        out=out_ap[:, qc : qc + 1, :],
                        in_=out_rows[:, qc : qc + 1, :],
                    )

        for _ in range(repeat):
            emit_once()

    nc.compile()
    return nc


def own_rows(half):
    """Query rows of a core: blocks {B0,B3} / {B1,B2} of 256 rows each.
    This interleaving is what makes the uniform SA_SCHED causally valid
    for both cores of a batch pair."""
    if half == 0:
        return np.concatenate([np.arange(0, 256), np.arange(768, 1024)])
    return np.arange(256, 768)


def _prep_core(c, x, kv, future_mask, mask, use_ca_mask):
    b, half = c // 2, c % 2
    own = own_rows(half)
    if half == 0:
        rest = np.concatenate([np.arange(256, 512), np.arange(512, 768)])
    else:
        rest = np.concatenate([np.arange(0, 256), np.arange(768, 1024)])
    perm = np.concatenate([own, rest])
    m = {}
    m["x_own"] = (np.ascontiguousarray(x[b, own][IDX4]) * OUT_SCALE).astype(np.float16)
    m["x_rm"] = np.ascontiguousarray(x[b][perm][IDX8]).astype(np.float16)
    m["kv_rm"] = np.ascontiguousarray(kv[b][IDX8]).astype(np.float16)
    # sa_cb[p, kc, q] = 0 where future_mask[b, own_q, perm_key] else 1 (key=kc*128+p)
    fm = future_mask[b, own][:, perm]  # [QN, S] bool
    cb = np.where(fm.T, np.float16(0.0), np.float16(1.0))  # [S, QN]
    m["sa_cb"] = np.ascontiguousarray(cb.reshape(RC, P, QN).transpose(1, 0, 2))
    if use_ca_mask:
        cm = mask[b, own]  # [QN, S]
        ccb = np.where(cm.T, np.float16(0.0), np.float16(1.0))
        m["ca_cb"] = np.ascontiguousarray(ccb.reshape(RC, P, QN).transpose(1, 0, 2))
    return m


def _q8w(w):
    """Host e4m3 quantization of a weight at WQ8 scale (clipped to range)."""
    return np.clip(np.asarray(w, np.float32) * WQ8, -240.0, 240.0).astype(E4NP)


def _hl8(w):
    """[H,H] -> [2H,H] fp8 [hi | lo]: hi = e4m3(32w), lo = e4m3(32w - hi).
    Interleaved so the device tile [P, 16, H] holds hi at k 0..7, lo at
    k 8..15 under the "(p k) n -> p k n" DMA rearrange."""
    w32 = np.ascontiguousarray(np.asarray(w)[IDX8]).astype(np.float32) * WQ8
    hi = np.clip(w32, -240.0, 240.0).astype(E4NP)
    lo = (w32 - hi.astype(np.float32)).astype(E4NP)
    return np.ascontiguousarray(
        np.concatenate(
            [hi.reshape(P, HT, H), lo.reshape(P, HT, H)], axis=1
        ).reshape(2 * H, H)
    )


def _prep_shared(inp):
    shared = {}
    shared["sa_wqhl"] = _hl8(inp["sa_wq"])
    shared["sa_wkhl"] = _hl8(inp["sa_wk"])
    shared["sa_wv"] = np.ascontiguousarray(
        np.asarray(inp["sa_wv"])[IDX8]
    ).astype(np.float16)
    shared["sa_wo"] = (
        np.ascontiguousarray(np.asarray(inp["sa_wo"])[IDX8]) * OUT_SCALE
    ).astype(np.float16)
    for w in ("ca_wq", "ca_wk", "ca_wv", "ca_wo"):
        shared[w] = _q8w(np.ascontiguousarray(np.asarray(inp[w])[IDX8]))
    shared["fc_w1"] = np.ascontiguousarray(
        np.asarray(inp["fc_w1"])[IDX8]
    ).astype(np.float16)
    shared["fc_w2"] = (
        np.ascontiguousarray(np.asarray(inp["fc_w2"])[IDX32]) * OUT_SCALE
    ).astype(np.float16)
    shared["fc_b1f"] = np.ascontiguousarray(
        np.asarray(inp["fc_b1"]).reshape(FT, P).T
    ).astype(np.float32)
    return shared


def kernel(**inputs) -> np.ndarray:
    global LAST_RUN_NS
    inp = {k: np.asarray(v) for k, v in inputs.items()}
    x, kv = inp["x"], inp["key_and_value"]
    mask, future_mask = inp["mask"], inp["future_mask"]

    flags = set()
    if mask.any():
        flags.add("ca_mask")
    # The clipped SA_SCHED structurally skips regions that a standard causal
    # mask guarantees are masked.  Only safe if future_mask IS causal triu;
    # otherwise fall back to the full-rectangle schedule (mask data covers it).
    tri = np.triu(np.ones((S, S), dtype=bool), 1)
    if not all(np.array_equal(future_mask[b], tri) for b in range(B)):
        flags.add("sa_full")
    for i in (1, 2, 3):
        if not np.all(inp[f"ln{i}_g"] == 1.0):
            flags.add(f"ln{i}_g")
        if np.any(inp[f"ln{i}_b"] != 0.0):
            flags.add(f"ln{i}_b")
    if np.any(inp["fc_b2"] != 0.0):
        flags.add("b2")
    flags = frozenset(flags)

    if flags not in _CACHE:
        _CACHE[flags] = _build(flags)
    nc = _CACHE[flags]

    shared = _prep_shared(inp)
    if "b2" in flags:
        shared["fc_b2"] = (inp["fc_b2"].reshape(1, H) * OUT_SCALE).astype(np.float32)
    for i in (1, 2, 3):
        if f"ln{i}_g" in flags:
            shared[f"ln{i}_g"] = inp[f"ln{i}_g"].reshape(1, H).astype(np.float32)
        if f"ln{i}_b" in flags:
            shared[f"ln{i}_b"] = inp[f"ln{i}_b"].reshape(1, H).astype(np.float32)

    in_maps = []
    for c in range(8):
        m = _prep_core(c, x, kv, future_mask, mask, "ca_mask" in flags)
        m.update(shared)
        in_maps.append(m)

    from concourse import bass_utils

    t0 = time.perf_counter_ns()
    res = bass_utils.run_bass_kernel_spmd(
        nc, in_maps, core_ids=list(range(8)), trace=False
    )
    LAST_RUN_NS = time.perf_counter_ns() - t0

    out = np.empty((B, S, H), np.float32)
    for c in range(8):
        b, half = c // 2, c % 2
        out[b, own_rows(half)] = res.results[c]["out"]
    out *= 1.0 / OUT_SCALE
    return out
